# revision 5
# baseline (speedup 1.0000x reference)
"""BiV-RWKV SpatialMix forward for Trainium2 (Bass/Tile), 8-core data-parallel.

Full inputs in, full output out. Internally: batch-shard B=16 over 8 cores
(2 per core), replicate the C x C weights.

Per-core program (C=192, T=3136):
  rows (b, c) packed into 3 partition tiles of 128:
    rt0 = (b0, c0:128), rt1 = (b0, c128:192)@rows0:64 + (b1, c128:192)@rows64:128,
    rt2 = (b1, c0:128)
  phase 1 (per row-tile, t ascending):  k,v,r projections on PE ->
    ek=exp(k), th=tanh(r/2) on ACT, ekv=ek*v on DVE,
    exclusive forward scans E,EQ via native tensor_tensor_scan (fp32 state)
  phase 2 (per row-tile, t descending): exclusive backward scans S,SQ
    (reversed-view scans), num/den = scalar_tensor_tensor with e^u,
    y = num * recip(den), h3 = (1+tanh)*(yf+yb)  [= 4*h, LN-invariant]
  phase 3 (t ascending): LN + output projection fused:
    col-sums of h3, h3^2 via ones-matmuls, A3 = 1/sqrt(var3+16eps),
    z = Wg@h3 + (-mu3) x wg  (rank-1 K=1 matmul into same PSUM group),
    out = A3 (x) z ; Wo@ln_b added on host.

The 64-row "hi" (c/d 128:192) chunks of b0 and b1 share one 128-partition
tile (b0 at rows 0:64, b1 at rows 64:128); the hi-half weights are loaded
twice (partition base 0 and 64) so matmul's lhsT.base == rhs.base holds.
"""
import os
os.environ.setdefault("JAX_PLATFORMS", "cpu")
import numpy as np
from contextlib import ExitStack

import concourse.bacc as bacc
import concourse.tile as tile
import concourse.mybir as mybir
from concourse.bass_utils import run_bass_kernel_spmd

f32 = mybir.dt.float32
Alu = mybir.AluOpType
Act = mybir.ActivationFunctionType

B, C, HH, WW = 16, 192, 56, 56
T = HH * WW            # 3136
NCORES = 8
BL = B // NCORES       # 2 batches per core
TT = 448
NJ = T // TT           # 7
LN_EPS = 1e-5
EPS16 = 16.0 * LN_EPS  # h3 = 4h  =>  var3 = 16 var
CH = 128               # lo chunk size
CR = C - CH            # 64, hi chunk size

_CACHE = {}


def _build_nc():
    nc = bacc.Bacc()
    xs = nc.dram_tensor("xs", [BL, C, T], f32, kind="ExternalInput")
    wkT = nc.dram_tensor("wkT", [C, C], f32, kind="ExternalInput")
    wvT = nc.dram_tensor("wvT", [C, C], f32, kind="ExternalInput")
    wrT = nc.dram_tensor("wrT", [C, C], f32, kind="ExternalInput")
    wgT = nc.dram_tensor("wgT", [C, C], f32, kind="ExternalInput")
    wgrow = nc.dram_tensor("wgrow", [1, C], f32, kind="ExternalInput")
    apack = nc.dram_tensor("apack", [3, 128], f32, kind="ExternalInput")
    eupack = nc.dram_tensor("eupack", [3, 128], f32, kind="ExternalInput")
    out = nc.dram_tensor("out", [BL, C, T], f32, kind="ExternalOutput")

    # (b, c_lo, c_hi, row_lo, row_hi) for each row tile
    RT = [
        [(0, 0, 128, 0, 128)],
        [(0, 128, 192, 0, 64), (1, 128, 192, 64, 128)],
        [(1, 0, 128, 0, 128)],
    ]

    with tile.TileContext(nc) as tc, ExitStack() as ctx:
        consts = ctx.enter_context(tc.tile_pool(name="consts", bufs=1))
        fullT = ctx.enter_context(tc.tile_pool(name="fullT", bufs=1))
        h3pool = ctx.enter_context(tc.tile_pool(name="h3pool", bufs=1))
        xin = ctx.enter_context(tc.tile_pool(name="xin", bufs=3))
        trans = ctx.enter_context(tc.tile_pool(name="trans", bufs=2))
        scpool = ctx.enter_context(tc.tile_pool(name="scpool", bufs=2))
        outsb = ctx.enter_context(tc.tile_pool(name="outsb", bufs=3))

        # ---- constants in SBUF ----
        # lo: W.T rows 0:128.  hi2: W.T rows 128:192 duplicated at partition
        # bases 0 and 64 (so lhsT base can match rhs base for packed-hi rhs).
        wt = {}
        for name, dram in (("k", wkT), ("v", wvT), ("r", wrT), ("g", wgT)):
            lo = consts.tile([CH, C], f32, name=f"w{name}lo", tag=f"w{name}lo")
            hi2 = consts.tile([128, C], f32, name=f"w{name}hi2", tag=f"w{name}hi2")
            nc.sync.dma_start(out=lo, in_=dram[0:CH, :])
            nc.sync.dma_start(out=hi2[0:64, :], in_=dram[CH:C, :])
            nc.sync.dma_start(out=hi2[64:128, :], in_=dram[CH:C, :])
            wt[name] = (lo, hi2)
        wg_sb = consts.tile([1, C], f32, name="wg_sb", tag="wg_sb")
        nc.sync.dma_start(out=wg_sb, in_=wgrow[:, :])
        ones_col = consts.tile([128, 1], f32, name="ones_col", tag="ones_col")
        nc.vector.memset(ones_col, 1.0)
        eps_t = consts.tile([1, 1], f32, name="eps_t", tag="eps_t")
        nc.vector.memset(eps_t, EPS16)
        a_t, eu_t = [], []
        for rt in range(3):
            at = consts.tile([128, 1], f32, name=f"a{rt}", tag=f"a{rt}")
            et = consts.tile([128, 1], f32, name=f"eu{rt}", tag=f"eu{rt}")
            nc.sync.dma_start(out=at,
                              in_=apack[rt, :].rearrange("(p one) -> p one", one=1))
            nc.sync.dma_start(out=et,
                              in_=eupack[rt, :].rearrange("(p one) -> p one", one=1))
            a_t.append(at)
            eu_t.append(et)

        # h3 layout mirrors the row tiles: h3_t[rt][:, t]
        h3_t = [h3pool.tile([128, T], f32, name=f"h3_{rt}", tag=f"h3_{rt}")
                for rt in range(3)]

        def x_tiles_for(rt, j):
            """DMA x slices this row tile's projections contract over.
            Returns {b: (xlo_tile, hi_pack_tile)}; hi rows of b live at
            partition base 64*b in the shared hi-pack tile."""
            js = j * TT
            bs = sorted({e[0] for e in RT[rt]})
            hi = xin.tile([128, TT], f32, name="xhi", tag="xhi")
            tiles = {}
            for b in bs:
                lo = xin.tile([128, TT], f32, name=f"xlo{b}", tag=f"xlo{b}")
                nc.sync.dma_start(out=lo, in_=xs[b, 0:CH, js:js + TT])
                nc.sync.dma_start(out=hi[64 * b:64 * b + 64, :],
                                  in_=xs[b, CH:C, js:js + TT])
                tiles[b] = (lo, hi)
            return tiles

        def proj(psum_t, w, rhs_tiles, rt):
            """psum_t[rows] = W-proj for this row tile's (b, d-range) rows."""
            lo, hi2 = w
            for (b, c_lo, c_hi, r_lo, r_hi) in RT[rt]:
                xlo, xhi = rhs_tiles[b]
                hb = 64 * b
                o = psum_t[r_lo:r_hi, :]
                nc.tensor.matmul(out=o, lhsT=lo[:, c_lo:c_hi], rhs=xlo,
                                 start=True, stop=False)
                nc.tensor.matmul(out=o, lhsT=hi2[hb:hb + 64, c_lo:c_hi],
                                 rhs=xhi[hb:hb + 64, :],
                                 start=False, stop=True)

        # ================= phases 1+2 per row tile =================
        with tc.tile_pool(name="pp12", bufs=2, space="PSUM") as pp12:
            for rt in range(3):
                ek = fullT.tile([128, T], f32, name="ek", tag="ek")
                ekv = fullT.tile([128, T], f32, name="ekv", tag="ekv")
                th = fullT.tile([128, T], f32, name="th", tag="th")
                E = fullT.tile([128, T], f32, name="E", tag="E")
                EQ = fullT.tile([128, T], f32, name="EQ", tag="EQ")
                a_bc448 = a_t[rt][:, 0:1].to_broadcast([128, TT])
                a_bc447 = a_t[rt][:, 0:1].to_broadcast([128, TT - 1])
                eu = eu_t[rt][:, 0:1]

                # ---- phase 1: ascending t ----
                for j in range(NJ):
                    js = j * TT
                    xt = x_tiles_for(rt, j)
                    kp = pp12.tile([128, TT], f32, name="kp", tag="kp")
                    proj(kp, wt["k"], xt, rt)
                    nc.scalar.activation(out=ek[:, js:js + TT], in_=kp,
                                         func=Act.Exp, bias=0.0, scale=1.0)
                    vp = pp12.tile([128, TT], f32, name="vp", tag="vp")
                    proj(vp, wt["v"], xt, rt)
                    nc.vector.tensor_mul(out=ekv[:, js:js + TT],
                                         in0=ek[:, js:js + TT], in1=vp)
                    rp = pp12.tile([128, TT], f32, name="rp", tag="rp")
                    proj(rp, wt["r"], xt, rt)
                    nc.scalar.activation(out=th[:, js:js + TT], in_=rp,
                                         func=Act.Tanh, bias=0.0, scale=0.5)
                    # exclusive forward scans: E_t = a*E_{t-1} + d_{t-1}
                    if j == 0:
                        nc.vector.memset(EQ[:, 0:1], 0.0)
                        nc.vector.memset(E[:, 0:1], 0.0)
                        nc.vector.tensor_tensor_scan(
                            out=EQ[:, 1:TT], data0=a_bc447, data1=ek[:, 0:TT - 1],
                            initial=0.0, op0=Alu.mult, op1=Alu.add)
                        nc.vector.tensor_tensor_scan(
                            out=E[:, 1:TT], data0=a_bc447, data1=ekv[:, 0:TT - 1],
                            initial=0.0, op0=Alu.mult, op1=Alu.add)
                    else:
                        nc.vector.tensor_tensor_scan(
                            out=EQ[:, js:js + TT], data0=a_bc448,
                            data1=ek[:, js - 1:js + TT - 1],
                            initial=EQ[:, js - 1:js], op0=Alu.mult, op1=Alu.add)
                        nc.vector.tensor_tensor_scan(
                            out=E[:, js:js + TT], data0=a_bc448,
                            data1=ekv[:, js - 1:js + TT - 1],
                            initial=E[:, js - 1:js], op0=Alu.mult, op1=Alu.add)

                # ---- phase 2: descending t ----
                SQ_next, S_next = None, None
                for j in range(NJ - 1, -1, -1):
                    js = j * TT
                    SQ = scpool.tile([128, TT], f32, name="SQ", tag="SQ")
                    S = scpool.tile([128, TT], f32, name="S", tag="S")
                    if j == NJ - 1:
                        nc.vector.memset(SQ[:, TT - 1:TT], 0.0)
                        nc.vector.memset(S[:, TT - 1:TT], 0.0)
                        nc.vector.tensor_tensor_scan(
                            out=SQ[:, 0:TT - 1][:, ::-1], data0=a_bc447,
                            data1=ek[:, js + 1:T][:, ::-1],
                            initial=0.0, op0=Alu.mult, op1=Alu.add)
                        nc.vector.tensor_tensor_scan(
                            out=S[:, 0:TT - 1][:, ::-1], data0=a_bc447,
                            data1=ekv[:, js + 1:T][:, ::-1],
                            initial=0.0, op0=Alu.mult, op1=Alu.add)
                    else:
                        nc.vector.tensor_tensor_scan(
                            out=SQ[:, ::-1], data0=a_bc448,
                            data1=ek[:, js + 1:js + TT + 1][:, ::-1],
                            initial=SQ_next[:, 0:1], op0=Alu.mult, op1=Alu.add)
                        nc.vector.tensor_tensor_scan(
                            out=S[:, ::-1], data0=a_bc448,
                            data1=ekv[:, js + 1:js + TT + 1][:, ::-1],
                            initial=S_next[:, 0:1], op0=Alu.mult, op1=Alu.add)
                    SQ_next, S_next = SQ, S

                    denf = trans.tile([128, TT], f32, name="denf", tag="denf")
                    denb = trans.tile([128, TT], f32, name="denb", tag="denb")
                    numf = trans.tile([128, TT], f32, name="numf", tag="numf")
                    numb = trans.tile([128, TT], f32, name="numb", tag="numb")
                    nc.vector.scalar_tensor_tensor(
                        out=denf, in0=ek[:, js:js + TT], scalar=eu,
                        in1=EQ[:, js:js + TT], op0=Alu.mult, op1=Alu.add)
                    nc.vector.scalar_tensor_tensor(
                        out=denb, in0=ek[:, js:js + TT], scalar=eu,
                        in1=SQ, op0=Alu.mult, op1=Alu.add)
                    nc.vector.scalar_tensor_tensor(
                        out=numf, in0=ekv[:, js:js + TT], scalar=eu,
                        in1=E[:, js:js + TT], op0=Alu.mult, op1=Alu.add)
                    nc.vector.scalar_tensor_tensor(
                        out=numb, in0=ekv[:, js:js + TT], scalar=eu,
                        in1=S, op0=Alu.mult, op1=Alu.add)
                    nc.vector.reciprocal_approx_fast(out=denf, in_=denf)
                    nc.vector.reciprocal_approx_fast(out=denb, in_=denb)
                    nc.vector.tensor_mul(out=numf, in0=numf, in1=denf)
                    nc.vector.tensor_mul(out=numb, in0=numb, in1=denb)
                    nc.vector.tensor_add(out=numf, in0=numf, in1=numb)
                    nc.vector.scalar_tensor_tensor(
                        out=h3_t[rt][:, js:js + TT], in0=th[:, js:js + TT],
                        scalar=1.0, in1=numf, op0=Alu.add, op1=Alu.mult)

        # ================= phase 3: LN + output projection =================
        invC = 1.0 / C
        with tc.tile_pool(name="pp3", bufs=2, space="PSUM") as pp3, \
             tc.tile_pool(name="zp3", bufs=4, space="PSUM") as zp3:
            for j in range(NJ):
                js = j * TT
                h3sq = []
                for rt in range(3):
                    hq = trans.tile([128, TT], f32, name=f"h3sq{rt}",
                                    tag=f"h3sq{rt}")
                    nc.gpsimd.tensor_mul(out=hq, in0=h3_t[rt][:, js:js + TT],
                                         in1=h3_t[rt][:, js:js + TT])
                    h3sq.append(hq)
                # col-sums at psum partitions: sum_b0@0, sum_b1@32,
                # sumsq_b0@64, sumsq_b1@96
                stats = pp3.tile([128, TT], f32, name="stats", tag="stats")
                chunks = [
                    (0, [(h3_t[0][:, js:js + TT], 0), (h3_t[1][0:64, js:js + TT], 0)]),
                    (32, [(h3_t[2][:, js:js + TT], 0), (h3_t[1][64:128, js:js + TT], 64)]),
                    (64, [(h3sq[0][:, :], 0), (h3sq[1][0:64, :], 0)]),
                    (96, [(h3sq[2][:, :], 0), (h3sq[1][64:128, :], 64)]),
                ]
                for row, rhss in chunks:
                    for i, (rhs, kb) in enumerate(rhss):
                        kdim = rhs.shape[0]
                        nc.tensor.matmul(out=stats[row:row + 1, :],
                                         lhsT=ones_col[kb:kb + kdim, 0:1], rhs=rhs,
                                         start=(i == 0), stop=(i == len(rhss) - 1),
                                         tile_position=(kb, row))
                # per-b LN stats: mu (ACT copy*1/C from PSUM), var3, A3
                A3b, mu_b = [], []
                for b in range(2):
                    mu = trans.tile([1, TT], f32, name=f"mu{b}", tag=f"mu{b}")
                    nc.scalar.activation(out=mu, in_=stats[32 * b:32 * b + 1, :],
                                         func=Act.Copy, bias=0.0, scale=invC)
                    mu_b.append(mu)
                    t1 = trans.tile([1, TT], f32, name=f"t1_{b}", tag=f"t1_{b}")
                    nc.vector.tensor_mul(out=t1, in0=mu, in1=mu)
                    v0 = trans.tile([1, TT], f32, name=f"v0_{b}", tag=f"v0_{b}")
                    nc.vector.scalar_tensor_tensor(
                        out=v0, in0=stats[64 + 32 * b:64 + 32 * b + 1, :],
                        scalar=invC, in1=t1, op0=Alu.mult, op1=Alu.subtract)
                    sqv = trans.tile([1, TT], f32, name=f"sqv{b}", tag=f"sqv{b}")
                    nc.scalar.activation(out=sqv, in_=v0, func=Act.Sqrt,
                                         bias=eps_t[0:1, 0:1], scale=1.0)
                    a3 = trans.tile([1, TT], f32, name=f"A3_{b}", tag=f"A3_{b}")
                    nc.vector.reciprocal_approx_fast(out=a3, in_=sqv)
                    A3b.append(a3)
                negmu = mu_b  # rank-1 lhsT is host-negated wg, so rhs is +mu
                A3bc = []
                for b in range(2):
                    ab = trans.tile([128, TT], f32, name=f"A3bc{b}", tag=f"A3bc{b}")
                    nc.gpsimd.partition_broadcast(ab, A3b[b][0:1, :])
                    A3bc.append(ab)
                # z = Wg @ h3 + (-mu3) x wg, then out = A3 (x) z
                glo, ghi2 = wt["g"]
                h3rhs = {0: (h3_t[0][:, js:js + TT], h3_t[1][0:64, js:js + TT]),
                         1: (h3_t[2][:, js:js + TT], h3_t[1][64:128, js:js + TT])}
                zp = []
                for rt in range(3):
                    zpt = zp3.tile([128, TT], f32, name=f"zp{rt}", tag="zp")
                    zp.append(zpt)
                for rt in range(3):
                    for (b, d_lo, d_hi, r_lo, r_hi) in RT[rt]:
                        o = zp[rt][r_lo:r_hi, :]
                        rlo, rhi = h3rhs[b]
                        hb = 64 * b
                        nc.tensor.matmul(out=o, lhsT=glo[:, d_lo:d_hi], rhs=rlo,
                                         start=True, stop=False)
                        nc.tensor.matmul(out=o, lhsT=ghi2[hb:hb + 64, d_lo:d_hi],
                                         rhs=rhi, start=False, stop=False,
                                         tile_position=(hb, r_lo))
                        nc.tensor.matmul(out=o, lhsT=wg_sb[0:1, d_lo:d_hi],
                                         rhs=negmu[b][0:1, :],
                                         start=False, stop=True)
                for rt in range(3):
                    ot = outsb.tile([128, TT], f32, name="ot", tag="ot")
                    for (b, d_lo, d_hi, r_lo, r_hi) in RT[rt]:
                        nc.vector.tensor_mul(out=ot[r_lo:r_hi, :],
                                             in0=zp[rt][r_lo:r_hi, :],
                                             in1=A3bc[b][r_lo:r_hi, :])
                        nc.sync.dma_start(
                            out=out[b, d_lo:d_hi, js:js + TT],
                            in_=ot[r_lo:r_hi, :])

    nc.compile()
    return nc


def kernel(x, Wk, Wv, Wr, Wo, ln_g, ln_b, w_decay, u):
    x = np.ascontiguousarray(x, np.float32)
    xs_full = x.reshape(B, C, T)

    a = np.exp(-np.exp(np.asarray(w_decay, np.float64))).astype(np.float32)
    eu = np.exp(np.asarray(u, np.float64)).astype(np.float32)
    apack = np.stack([a[0:128],
                      np.concatenate([a[128:192], a[128:192]]),
                      a[0:128]]).astype(np.float32)
    eupack = np.stack([eu[0:128],
                       np.concatenate([eu[128:192], eu[128:192]]),
                       eu[0:128]]).astype(np.float32)
    Wg = (np.asarray(Wo) * np.asarray(ln_g)[None, :]).astype(np.float32)
    consts = {
        "wkT": np.ascontiguousarray(np.asarray(Wk).T, np.float32),
        "wvT": np.ascontiguousarray(np.asarray(Wv).T, np.float32),
        "wrT": np.ascontiguousarray(np.asarray(Wr).T, np.float32),
        "wgT": np.ascontiguousarray(Wg.T, np.float32),
        # negated so the rank-1 matmul adds (-mu3) x wg with rhs = +mu3
        "wgrow": np.ascontiguousarray(-(np.asarray(Wo) @ np.asarray(ln_g))[None, :],
                                      np.float32),
        "apack": apack, "eupack": eupack,
    }

    if "nc" not in _CACHE:
        _CACHE["nc"] = _build_nc()
    nc = _CACHE["nc"]

    in_maps = [dict(xs=np.ascontiguousarray(xs_full[i * BL:(i + 1) * BL]),
                    **consts) for i in range(NCORES)]
    res = run_bass_kernel_spmd(nc, in_maps, core_ids=list(range(NCORES)),
                               **_CACHE.get("run_kwargs", {}))
    _CACHE["last_res"] = res
    out = np.stack([r["out"] for r in res.results]).reshape(B, C, T)
    wb = (np.asarray(Wo) @ np.asarray(ln_b)).astype(np.float32)
    out = out + wb[None, :, None]
    return out.reshape(B, C, HH, WW).astype(np.float32)
